# revision 13
# baseline (speedup 1.0000x reference)
"""Chamfer distance (K=1 squared-euclidean NN, both directions) on 8
Trainium2 NeuronCores.

Sharding: 8 independent work units = 4 batches x 2 directions; one unit per
core (SPMD — same program, different inputs). Per unit: queries Q[8192,3]
vs keys K[8192,3].

Device algorithm per unit (raw Bass, explicit semaphores):
  u[p,q] = 2*q_p.k_q - ||q_p||^2 - ||k_q||^2 = -(d^2)     via K=5 matmul
    with augmented operands lhsT = [2qx,2qy,2qz,||q||^2,1],
                            rhs  = [kx,ky,kz,-1,-||k||^2].
  * TensorE: 16 concurrent tile_position matmuls (4 row-groups x 4
    col-groups, K=5, M=32, N=512) fill a [128,2048] 4-bank PSUM supertile;
    two supertiles ping-pong.
  * VectorE: one 1x-rate pass total — a running prefix-max scan
    (tensor_tensor_scan, op=max, carry chained across supertiles) PSUM ->
    SBUF. M = last scan element = exact global max of u; cham = -M.
  * ScalarE: argmax (first occurrence) via a counting trick: over the scan
    output, sign(scan_q - M) is -1 exactly for q < q_first and 0 after, so
    idx = -sum_q sign(scan_q - M), which activation(Sign, bias=-M,
    accum_out) yields in one pass. Integer sums < 2^24 -> exact in fp32.

The DVE pass is the bottleneck (~1 elem/lane/cycle @0.96GHz); PE and ACT
hide underneath it.
"""

import numpy as np

import concourse.bass as bass
import concourse.mybir as mybir
from concourse.bass_utils import run_bass_kernel_spmd

F32 = mybir.dt.float32
NEG_BIG = -3.0e38

N_BATCH = 4
NPTS = 8192
N_CORES = 8


def build_chamfer_bass(P1=NPTS, P2=NPTS, sup=2048, repeat=1):
    """Single-core Bass program (SPMD across cores).

    P1: query count (blocked by 128), P2: key count (streamed in `sup`-wide
    supertiles; sup*4B <= 8KB -> 4 PSUM banks).
    repeat: run the whole computation `repeat` times back-to-back inside one
    NEFF (benchmarking only — wall-clock slope isolates device time)."""
    assert P1 % 128 == 0 and P2 % sup == 0 and sup % 512 == 0
    qb = P1 // 128
    nsup = P2 // sup
    ntile = sup // 512
    ngroups = qb * nsup
    assert ntile <= 4

    nc = bass.Bass()
    # qa rows 0..4 in columns [0,P1); ka rows 0..4 in columns [P1,P1+P2).
    qka = nc.dram_tensor("qka", [5, P1 + P2], F32, kind="ExternalInput")
    cham = nc.dram_tensor("cham", [128, qb], F32, kind="ExternalOutput")
    idx = nc.dram_tensor("idx", [128, qb], mybir.dt.int32, kind="ExternalOutput")

    with (
        nc.sbuf_tensor([128, P1 + P2], F32) as qka_sb,
        nc.sbuf_tensor([128, sup], F32) as dummy,
        nc.sbuf_tensor([128, P2], F32) as scan0,
        nc.sbuf_tensor([128, P2], F32) as scan1,
        nc.sbuf_tensor([128, P2], mybir.dt.bfloat16) as junk,
        nc.sbuf_tensor([128, qb], F32) as negm,
        nc.sbuf_tensor([128, qb], F32) as acc,
        nc.sbuf_tensor([128, qb], mybir.dt.int32) as idx_i,
        nc.psum_tensor([128, sup], F32) as ps_a,
        nc.psum_tensor([128, sup], F32) as ps_b,
        nc.semaphore("s_dma") as s_dma,
        nc.semaphore("s_mm") as s_mm,
        nc.semaphore("s_scan") as s_scan,
        nc.semaphore("s_act") as s_act,
        nc.semaphore("s_ext") as s_ext,
        nc.semaphore("s_fin") as s_fin,
        nc.Block() as block,
    ):
        ps = [ps_a, ps_b]
        scanbufs = [scan0, scan1]

        @block.sync
        def _(sync):
            for i in range(4):
                sync.dma_start(
                    out=qka_sb[32 * i:32 * i + 5, :], in_=qka[:, :]
                ).then_inc(s_dma, 16)
            # dummy operand for the scan's ignored (bypass) data1 stream —
            # only needs to be *initialized*; values are irrelevant.
            sync.dma_start(
                out=dummy[:, :],
                in_=bass.AP(tensor=qka, offset=0, ap=[[0, 128], [1, sup]]),
            ).then_inc(s_dma, 16)
            # outputs
            sync.wait_ge(s_act, qb * repeat)
            sync.dma_start(out=cham[:, :], in_=negm[:, :]).then_inc(s_dma, 16)
            sync.wait_ge(s_fin, 1)
            sync.dma_start(out=idx[:, :], in_=idx_i[:, :]).then_inc(s_dma, 16)
            sync.wait_ge(s_dma, 16 * 7)

        @block.tensor
        def _(tensor):
            tensor.wait_ge(s_dma, 16 * 5)
            g = 0
            for m in range(qb * repeat):
                m = m % qb
                for s in range(nsup):
                    if g >= 2:
                        tensor.wait_ge(s_scan, g - 1)
                    pst = ps[g % 2]
                    last = None
                    for i in range(ntile):
                        for j in range(4):
                            last = nc.tensor.matmul(
                                pst[32 * j:32 * j + 32, 512 * i:512 * (i + 1)],
                                qka_sb[32 * i:32 * i + 5,
                                       128 * m + 32 * j:128 * m + 32 * j + 32],
                                qka_sb[32 * i:32 * i + 5,
                                       P1 + sup * s + 512 * i:
                                       P1 + sup * s + 512 * (i + 1)],
                                start=True, stop=True,
                                tile_position=(32 * i, 32 * j),
                            )
                    last.then_inc(s_mm, 1)
                    g += 1

        @block.vector
        def _(vector):
            vector.wait_ge(s_dma, 16 * 5)
            g = 0
            for mb in range(qb * repeat):
                m = mb % qb
                sb = scanbufs[mb % 2]
                for s in range(nsup):
                    vector.wait_ge(s_mm, g + 1)
                    if s == 0 and mb >= 2:
                        vector.wait_ge(s_act, mb - 1)
                    if s > 0:
                        # order the carry-element read after the previous
                        # scan's committed write (same-engine; makes the
                        # happens-before explicit)
                        vector.wait_ge(s_scan, g)
                    init = NEG_BIG if s == 0 else sb[:, sup * s - 1:sup * s]
                    nc.vector.tensor_tensor_scan(
                        out=sb[:, sup * s:sup * (s + 1)],
                        data0=ps[g % 2][:, :],
                        data1=dummy[:, :],
                        initial=init,
                        op0=mybir.AluOpType.max,
                        op1=mybir.AluOpType.bypass,
                    ).then_inc(s_scan, 1)
                    g += 1
            vector.wait_ge(s_act, qb * repeat)
            nc.vector.tensor_scalar(
                out=idx_i[:, :], in0=acc[:, :], scalar1=-1.0, scalar2=None,
                op0=mybir.AluOpType.mult,
            ).then_inc(s_fin, 1)

        @block.scalar
        def _(scalar):
            for mb in range(qb * repeat):
                m = mb % qb
                scalar.wait_ge(s_scan, nsup * (mb + 1))
                sb = scanbufs[mb % 2]
                nc.scalar.activation(
                    out=negm[:, m:m + 1], in_=sb[:, P2 - 1:P2],
                    func=mybir.ActivationFunctionType.Copy, scale=-1.0,
                ).then_inc(s_ext, 1)
                # explicit same-engine ordering: bias read after extract's
                # committed write; junk WAW after the previous sign
                scalar.wait_ge(s_ext, mb + 1)
                if mb >= 1:
                    scalar.wait_ge(s_act, mb)
                nc.scalar.activation(
                    out=junk[:, :], in_=sb[:, :],
                    func=mybir.ActivationFunctionType.Sign,
                    bias=negm[:, m:m + 1], scale=1.0,
                    accum_out=acc[:, m:m + 1],
                ).then_inc(s_act, 1)

    return nc


def make_unit_inputs(q, k):
    """Host-side augmentation for one (query cloud, key cloud) unit."""
    q = np.ascontiguousarray(q, np.float32)
    k = np.ascontiguousarray(k, np.float32)
    p1, p2 = q.shape[0], k.shape[0]
    qka = np.empty((5, p1 + p2), np.float32)
    qka[0:3, :p1] = 2.0 * q.T
    qka[3, :p1] = (q * q).sum(-1, dtype=np.float32)
    qka[4, :p1] = 1.0
    qka[0:3, p1:] = k.T
    qka[3, p1:] = -1.0
    qka[4, p1:] = -((k * k).sum(-1, dtype=np.float32))
    return {"qka": qka}


_BUILT = {}


def _built_nc():
    if "nc" not in _BUILT:
        _BUILT["nc"] = build_chamfer_bass()
    return _BUILT["nc"]


def kernel(x, y, _collect_results=None):
    """Full-input entry point. x, y: (4, 8192, 3) float32.

    Returns (cham_x, cham_y, idx_x, idx_y) matching reference()."""
    x = np.asarray(x, np.float32)
    y = np.asarray(y, np.float32)
    n = x.shape[0]
    units = []
    in_maps = []
    for b in range(n):
        for d in range(2):
            q, k = (x[b], y[b]) if d == 0 else (y[b], x[b])
            in_maps.append(make_unit_inputs(q, k))
            units.append((b, d))
    nc = _built_nc()
    res = run_bass_kernel_spmd(nc, in_maps, core_ids=list(range(N_CORES)))
    if _collect_results is not None:
        _collect_results.append(res)
    cham_x = np.empty((n, x.shape[1]), np.float32)
    cham_y = np.empty((n, y.shape[1]), np.float32)
    idx_x = np.empty((n, x.shape[1]), np.int32)
    idx_y = np.empty((n, y.shape[1]), np.int32)
    for (b, d), r in zip(units, res.results):
        chamv = np.asarray(r["cham"]).T.reshape(-1)
        idxv = np.asarray(r["idx"]).T.reshape(-1)
        if d == 0:
            cham_x[b], idx_x[b] = chamv, idxv
        else:
            cham_y[b], idx_y[b] = chamv, idxv
    return cham_x, cham_y, idx_x, idx_y
